# revision 1
# baseline (speedup 1.0000x reference)
"""Trainium2 Bass kernel for MultiInterestExtractor (matmul + gumbel softmax + top-10).

Data-parallel over batch across 8 cores. Per core: 512*200=102400 tokens.
Per 128-token tile:
  PE:  transpose x-tile -> xT (psum), z = xT.T @ C^T (+ gumbel noise via
       identity-matmul accumulate) in PSUM
  ACT: copy xT psum->sbuf; e = Exp(z * 1/tau) with accum -> row sum s
  DVE: top-8 (max), indices (max_index), match_replace, top-8 again for
       ranks 9..10, reciprocal + scale by 1/s
"""

import numpy as np

import concourse.bass as bass
import concourse.mybir as mybir
import concourse.tile as tile_mod
from concourse.tile import TileContext
from concourse.vector_clock import ScopedClock

B, L, H, A = 4096, 200, 64, 64
TAU = 10.0
K = 10
NCORES = 8
TOK = B * L // NCORES          # 102400 tokens per core
TILE = 128                     # tokens per tile (partition dim)
SUPER = 8                      # tiles per DMA batch
NTILES = TOK // TILE           # 800
NSUPER = NTILES // SUPER       # 100

_MAX_WAITS = 1


def _patched_drain_and_barrier(self, tick_clock, wait_clock):
    # core_v3 codegen allows only 1 sem wait per Drain: spread the tail
    # drain's global-clock waits over several drain instructions.
    nc = self.nc
    drain_inst = nc.sync.drain()
    wait_clock.add_sem_waits(
        drain_inst.ins, ScopedClock({None: tick_clock.global_clock})
    )
    si = drain_inst.ins.sync_info
    waits = list(si.on_wait or [])
    if len(waits) > _MAX_WAITS:
        si.on_wait = waits[:_MAX_WAITS]
        rest = waits[_MAX_WAITS:]
        while rest:
            extra = nc.sync.drain()
            extra.ins.sync_info = mybir.SyncInfo(
                on_wait=rest[:_MAX_WAITS], on_update=[]
            )
            rest = rest[_MAX_WAITS:]
    nc.all_engine_barrier()
    assert self.sems is not None
    popped = nc._tile_sem_poison_stack.pop()
    assert popped is self._sem_poison
    nc.clear_and_free_semaphores(list(self.sems.allocated().values()))
    nc.all_engine_barrier()


tile_mod.TileContext._drain_and_barrier = _patched_drain_and_barrier

_orig_commit = tile_mod.TileContext._commit_instruction


def _patched_commit(self, inst, lazy_reg_writes=True):
    # core_v3 codegen allows only 1 sem wait per instruction on this build:
    # peel extra waits onto same-engine Drain carriers committed just before.
    si = inst.sync_info
    if (
        si is not None
        and si.on_wait
        and len(si.on_wait) > _MAX_WAITS
        and inst.engine != mybir.EngineType.Unassigned
    ):
        waits = list(si.on_wait)
        keep = waits[-_MAX_WAITS:]
        rest = waits[:-_MAX_WAITS]
        while rest:
            carrier = mybir.InstDrain(
                name=f"I-{self.nc.next_id()}",
                engine=inst.engine,
                sync_info=mybir.SyncInfo(
                    on_wait=rest[:_MAX_WAITS], on_update=[]
                ),
            )
            rest = rest[_MAX_WAITS:]
            self._add_instruction(carrier)
        si.on_wait = keep
    return _orig_commit(self, inst, lazy_reg_writes)


tile_mod.TileContext._commit_instruction = _patched_commit

_CACHED = {}


def build():
    if "nc" in _CACHED:
        return _CACHED["nc"]
    f32 = mybir.dt.float32
    nc = bass.Bass()
    x = nc.dram_tensor("x", [TOK, H], f32, kind="ExternalInput")
    g = nc.dram_tensor("g", [TOK, A], f32, kind="ExternalInput")
    ct = nc.dram_tensor("ct", [H, A], f32, kind="ExternalInput")
    ident = nc.dram_tensor("ident", [TILE, TILE], f32, kind="ExternalInput")
    wout = nc.dram_tensor("wout", [TOK, K], f32, kind="ExternalOutput")
    iout = nc.dram_tensor("iout", [TOK, K], mybir.dt.uint32, kind="ExternalOutput")
    sout = nc.dram_tensor("sout", [TOK, 1], f32, kind="ExternalOutput")

    with TileContext(nc) as tc:
        with tc.tile_pool(name="singles", bufs=1) as singles, \
             tc.tile_pool(name="xg", bufs=3) as xg, \
             tc.tile_pool(name="xt", bufs=4) as xtp, \
             tc.tile_pool(name="ep", bufs=4) as ep, \
             tc.tile_pool(name="small", bufs=6) as small, \
             tc.tile_pool(name="outs", bufs=3) as outs, \
             tc.tile_pool(name="ps_t", bufs=2, space="PSUM") as ps_t, \
             tc.tile_pool(name="ps_z", bufs=4, space="PSUM") as ps_z:

            ct_sb = singles.tile([H, A], f32)
            nc.sync.dma_start(out=ct_sb, in_=ct[:, :])
            id_sb = singles.tile([TILE, TILE], f32)
            nc.sync.dma_start(out=id_sb, in_=ident[:, :])

            for s in range(NSUPER):
                t0 = s * SUPER * TILE
                xs = xg.tile([TILE, SUPER, H], f32)
                nc.sync.dma_start(
                    out=xs,
                    in_=x[t0:t0 + SUPER * TILE, :].rearrange(
                        "(j p) h -> p j h", p=TILE),
                )
                gs = xg.tile([TILE, SUPER, A], f32)
                nc.sync.dma_start(
                    out=gs,
                    in_=g[t0:t0 + SUPER * TILE, :].rearrange(
                        "(j p) h -> p j h", p=TILE),
                )
                vsup = outs.tile([TILE, SUPER, 16], f32)
                isup = outs.tile([TILE, SUPER, 16], mybir.dt.uint32)
                ssup = outs.tile([TILE, SUPER, 1], f32)

                for j in range(SUPER):
                    xt_ps = ps_t.tile([H, TILE], f32)
                    nc.tensor.transpose(xt_ps, xs[:, j, :], id_sb)
                    xt_sb = xtp.tile([H, TILE], f32)
                    nc.scalar.activation(
                        out=xt_sb, in_=xt_ps,
                        func=mybir.ActivationFunctionType.Copy)
                    z_ps = ps_z.tile([TILE, A], f32)
                    nc.tensor.matmul(z_ps, lhsT=xt_sb, rhs=ct_sb,
                                     start=True, stop=False,
                                     skip_group_check=True)
                    nc.tensor.matmul(z_ps, lhsT=id_sb, rhs=gs[:, j, :],
                                     start=False, stop=True,
                                     skip_group_check=True)
                    e = ep.tile([TILE, A], f32)
                    nc.scalar.activation(
                        out=e, in_=z_ps,
                        func=mybir.ActivationFunctionType.Exp,
                        scale=1.0 / TAU, accum_out=ssup[:, j, :])
                    v16 = vsup[:, j, :]
                    i16 = isup[:, j, :]
                    nc.vector.max(out=v16[:, 0:8], in_=e)
                    nc.vector.max_index(out=i16[:, 0:8], in_max=v16[:, 0:8],
                                        in_values=e)
                    e2 = ep.tile([TILE, A], f32)
                    nc.vector.match_replace(out=e2, in_to_replace=v16[:, 0:8],
                                            in_values=e, imm_value=-1.0)
                    nc.vector.max(out=v16[:, 8:16], in_=e2)
                    nc.vector.max_index(out=i16[:, 8:16], in_max=v16[:, 8:16],
                                        in_values=e2)

                nc.sync.dma_start(
                    out=wout[t0:t0 + SUPER * TILE, :].rearrange(
                        "(j p) k -> p j k", p=TILE),
                    in_=vsup[:, :, 0:K],
                )
                nc.sync.dma_start(
                    out=iout[t0:t0 + SUPER * TILE, :].rearrange(
                        "(j p) k -> p j k", p=TILE),
                    in_=isup[:, :, 0:K],
                )
                nc.sync.dma_start(
                    out=sout[t0:t0 + SUPER * TILE, :].rearrange(
                        "(j p) k -> p j k", p=TILE),
                    in_=ssup,
                )
    _CACHED["nc"] = nc
    return nc


def run(x_u, C_weight, gumbel_noise, trace=False):
    from concourse.bass_utils import run_bass_kernel_spmd

    nc = build()
    xf = np.ascontiguousarray(x_u, dtype=np.float32).reshape(B * L, H)
    gf = np.ascontiguousarray(gumbel_noise, dtype=np.float32).reshape(B * L, A)
    ct = np.ascontiguousarray(np.asarray(C_weight, dtype=np.float32).T)
    ident = np.eye(TILE, dtype=np.float32)
    in_maps = []
    for c in range(NCORES):
        in_maps.append({
            "x": xf[c * TOK:(c + 1) * TOK],
            "g": gf[c * TOK:(c + 1) * TOK],
            "ct": ct,
            "ident": ident,
        })
    res = run_bass_kernel_spmd(nc, in_maps, core_ids=list(range(NCORES)),
                               trace=trace)
    w = np.concatenate([res.results[c]["wout"] for c in range(NCORES)], axis=0)
    i = np.concatenate([res.results[c]["iout"] for c in range(NCORES)], axis=0)
    s = np.concatenate([res.results[c]["sout"] for c in range(NCORES)], axis=0)
    w = w / s  # softmax denominator, applied on host
    w = w.reshape(B, L, K)
    i = i.view(np.int32).reshape(B, L, K)
    return (w, i), res


def kernel(x_u, C_weight, gumbel_noise):
    (w, i), _ = run(x_u, C_weight, gumbel_noise)
    return w, i



# revision 4
# speedup vs baseline: 1.9886x; 1.9886x over previous
"""Trainium2 Bass kernel for MultiInterestExtractor (matmul + gumbel
softmax + top-10 of 64 aspects).

Per core: 102400 tokens, processed in 100 supergroups of 1024 tokens
(8 tiles x 128 tokens, p-major token order: tok = s*1024 + p*8 + j).
x is pre-transposed on the host so no PE transposes / ACT copies are
needed; DMA descriptors are 2-4KB contiguous per partition.

Per supergroup:
  PE : 1 batched fp32 identity matmul routes gumbel into the PSUM bank
       (start=True over the whole bank), then 8 z-matmuls (K=64)
       accumulate x@C^T into the per-tile slots.
  ACT: 1 batched exp((z+g)/tau) [128,512] psum->sbuf, plus 1 broadcast
       copy materializing the per-tile top-8 threshold T.
  DVE: 1 batched bit-stuff  b = (e & ~63) | (63-a)  (uint32 ALU ops;
       encodes the aspect index in the low 6 mantissa bits while the
       values stay ordered as positive floats), 1 batched row-sum
       reduce, 8x max8 (top-8 with indices in the low bits), 1 batched
       is_ge + 1 batched fused mul-add suppressing the top-8 to
       -FLT_MAX, 8x max8 again giving ranks 9-16.
  Out: one packed [128, 8, 17] tile per super: top-8 stuffed f32,
       ranks 9-16 stuffed f32, row sum. Host decodes indices/weights
       (idx = 63 - (bits & 63), w = (bits & ~63) / rowsum).
"""

import numpy as np

import concourse.bass as bass
import concourse.mybir as mybir
import concourse.tile as tile_mod
from concourse.tile import TileContext
from concourse.vector_clock import ScopedClock

B, L, H, A = 4096, 200, 64, 64
TAU = 10.0
K = 10
NCORES = 8
TOK = B * L // NCORES          # 102400
TILE = 128
SUPER = 8                      # tiles per supergroup (one PSUM bank)
SUPTOK = SUPER * TILE          # 1024
NSUPER = TOK // SUPTOK         # 100
NPAIR = SUPER // 2

f32 = mybir.dt.float32
u32 = mybir.dt.uint32

SUPPRESS_ON_GPSIMD = False
# "usub": uint32 wraparound subtract (1 op; ranks 9-16 come out T-shifted)
# "fmask": f32 is_ge mask + fma with -FLT_MAX (2 ops; values come out raw)
SUPPRESS_MODE = "fmask"
PACK_PAIRS = False

_MAX_WAITS = 1


def _patched_drain_and_barrier(self, tick_clock, wait_clock):
    nc = self.nc
    drain_inst = nc.sync.drain()
    wait_clock.add_sem_waits(
        drain_inst.ins, ScopedClock({None: tick_clock.global_clock})
    )
    si = drain_inst.ins.sync_info
    waits = list(si.on_wait or [])
    if len(waits) > _MAX_WAITS:
        si.on_wait = waits[:_MAX_WAITS]
        rest = waits[_MAX_WAITS:]
        while rest:
            extra = nc.sync.drain()
            extra.ins.sync_info = mybir.SyncInfo(
                on_wait=rest[:_MAX_WAITS], on_update=[]
            )
            rest = rest[_MAX_WAITS:]
    nc.all_engine_barrier()
    assert self.sems is not None
    popped = nc._tile_sem_poison_stack.pop()
    assert popped is self._sem_poison
    nc.clear_and_free_semaphores(list(self.sems.allocated().values()))
    nc.all_engine_barrier()


tile_mod.TileContext._drain_and_barrier = _patched_drain_and_barrier

if not hasattr(tile_mod.TileContext, "_ant_orig_commit"):
    tile_mod.TileContext._ant_orig_commit = \
        tile_mod.TileContext._commit_instruction
_orig_commit = tile_mod.TileContext._ant_orig_commit


def _patched_commit(self, inst, lazy_reg_writes=True):
    si = inst.sync_info
    if (
        si is not None
        and si.on_wait
        and len(si.on_wait) > _MAX_WAITS
        and inst.engine != mybir.EngineType.Unassigned
    ):
        waits = list(si.on_wait)
        keep = waits[-_MAX_WAITS:]
        rest = waits[:-_MAX_WAITS]
        while rest:
            carrier = mybir.InstDrain(
                name=f"I-{self.nc.next_id()}",
                engine=inst.engine,
                sync_info=mybir.SyncInfo(
                    on_wait=rest[:_MAX_WAITS], on_update=[]
                ),
            )
            rest = rest[_MAX_WAITS:]
            self._add_instruction(carrier)
        si.on_wait = keep
    return _orig_commit(self, inst, lazy_reg_writes)


tile_mod.TileContext._commit_instruction = _patched_commit


def stt_u32(eng, out, in0, imm, in1, op0, op1):
    """scalar_tensor_tensor with a uint32-typed immediate (walrus requires
    an integer ImmVal matching src/dst dtype for bitvec ops)."""
    return eng.add_instruction(
        mybir.InstTensorScalarPtr(
            name=eng.bass.get_next_instruction_name(),
            is_scalar_tensor_tensor=True,
            op0=op0,
            op1=op1,
            ins=[
                eng.lower_ap(in0),
                mybir.ImmediateValue(dtype=u32, value=imm),
                eng.lower_ap(in1),
            ],
            outs=[eng.lower_ap(out)],
        )
    )


_CACHED = {}


def build():
    if "nc" in _CACHED:
        return _CACHED["nc"]
    nc = bass.Bass()
    xt = nc.dram_tensor("xt", [NSUPER, 64, SUPER * 128], f32,
                        kind="ExternalInput")
    g = nc.dram_tensor("g", [TOK, A], f32, kind="ExternalInput")
    ct2 = nc.dram_tensor("ct2", [64, A], f32, kind="ExternalInput")
    idp = nc.dram_tensor("idp", [128, SUPER * A], u32, kind="ExternalInput")
    ident = nc.dram_tensor("ident", [128, 128], f32, kind="ExternalInput")
    out = nc.dram_tensor("out", [NSUPER, 128, SUPER, 17], f32,
                         kind="ExternalOutput")

    with TileContext(nc) as tc:
        with tc.tile_pool(name="singles", bufs=1) as singles, \
             tc.tile_pool(name="ins", bufs=3) as ins, \
             tc.tile_pool(name="mid", bufs=3) as mid, \
             tc.tile_pool(name="outs", bufs=3) as outs, \
             tc.tile_pool(name="ps_z", bufs=3, space="PSUM") as ps_z:

            ct2_sb = singles.tile([64, A], f32)
            nc.sync.dma_start(out=ct2_sb, in_=ct2[:, :])
            id_sb = singles.tile([128, 128], f32)
            nc.sync.dma_start(out=id_sb, in_=ident[:, :])
            idp_sb = singles.tile([128, SUPER * A], u32)
            nc.sync.dma_start(out=idp_sb, in_=idp[:, :])

            for s in range(NSUPER):
                t0 = s * SUPTOK
                xt_sb = ins.tile([64, SUPER, 128], f32)
                nc.sync.dma_start(out=xt_sb,
                                  in_=xt[s].rearrange("h (j q) -> h j q",
                                                      j=SUPER))
                g_sb = ins.tile([128, SUPER, A], f32)
                nc.scalar.dma_start(
                    out=g_sb,
                    in_=g[t0:t0 + SUPTOK, :].rearrange("(p j) a -> p j a",
                                                       p=128),
                )

                zb = ps_z.tile([128, SUPER, A], f32)
                zb_flat = zb[:, :, :].rearrange("p j a -> p (j a)")
                nc.tensor.matmul(zb_flat, lhsT=id_sb,
                                 rhs=g_sb[:, :, :].rearrange(
                                     "p j a -> p (j a)"),
                                 start=True, stop=False,
                                 skip_group_check=True)
                for j in range(SUPER):
                    nc.tensor.matmul(zb[:, j, :],
                                     lhsT=xt_sb[:, j, :],
                                     rhs=ct2_sb,
                                     start=False, stop=True,
                                     skip_group_check=True)

                e_sb = mid.tile([128, SUPER, A], f32)
                nc.scalar.activation(out=e_sb, in_=zb,
                                     func=mybir.ActivationFunctionType.Exp,
                                     scale=1.0 / TAU)

                o_sb = outs.tile([128, SUPER, 17], f32)
                b_sb = mid.tile([128, SUPER, A], f32)
                stt_u32(
                    nc.vector,
                    out=b_sb[:, :, :].rearrange("p j a -> p (j a)").bitcast(u32),
                    in0=e_sb[:, :, :].rearrange("p j a -> p (j a)").bitcast(u32),
                    imm=int(0xFFFFFFC0),
                    in1=idp_sb[:, :],
                    op0=mybir.AluOpType.bitwise_and,
                    op1=mybir.AluOpType.bitwise_or,
                )
                nc.vector.tensor_reduce(
                    out=o_sb[:, :, 16],
                    in_=e_sb[:, :, :],
                    axis=mybir.AxisListType.X,
                    op=mybir.AluOpType.add,
                )
                for j in range(SUPER):
                    nc.vector.max(out=o_sb[:, j, 0:8], in_=b_sb[:, j, :])

                # materialize the per-tile threshold T=v8 broadcast along the
                # aspect dim (stride-0 AP reads misbehave as tensor_tensor
                # inputs on HW; an ACT broadcast-copy is cheap and proven)
                tmat_sb = mid.tile([128, SUPER, A], f32)
                nc.scalar.activation(
                    out=tmat_sb,
                    in_=o_sb[:, :, 7:8].broadcast_to([128, SUPER, A]),
                    func=mybir.ActivationFunctionType.Copy)
                sub_sb = mid.tile([128, SUPER, A], f32)
                sup_eng = nc.gpsimd if SUPPRESS_ON_GPSIMD else nc.vector
                if SUPPRESS_MODE == "usub":
                    sup_eng.tensor_tensor(
                        out=sub_sb[:, :, :].bitcast(u32),
                        in0=b_sb[:, :, :].bitcast(u32),
                        in1=tmat_sb[:, :, :].bitcast(u32),
                        op=mybir.AluOpType.subtract,
                    )
                    for j in range(SUPER):
                        nc.vector.max(out=o_sb[:, j, 8:16].bitcast(u32),
                                      in_=sub_sb[:, j, :].bitcast(u32))
                else:
                    mask_sb = mid.tile([128, SUPER, A], f32)
                    sup_eng.tensor_tensor(
                        out=mask_sb, in0=b_sb, in1=tmat_sb,
                        op=mybir.AluOpType.is_ge)
                    sup_eng.scalar_tensor_tensor(
                        out=sub_sb, in0=mask_sb, scalar=-3.4e38, in1=b_sb,
                        op0=mybir.AluOpType.mult, op1=mybir.AluOpType.add)
                    for j in range(SUPER):
                        nc.vector.max(out=o_sb[:, j, 8:16],
                                      in_=sub_sb[:, j, :])

                nc.gpsimd.dma_start(out=out[s], in_=o_sb)

    _CACHED["nc"] = nc
    return nc


def _prep_core_inputs(xc, gc, ct2, idp, ident):
    """xc: [TOK, H] core slice of x; returns the input map for one core."""
    # transposed x, p-major tokens: xt[s, h, j*128+p] = x[s*1024+p*8+j, h]
    xs = xc.reshape(NSUPER, 128, SUPER, H)            # [s, p, j, h]
    xt = np.ascontiguousarray(
        xs.transpose(0, 3, 2, 1).reshape(NSUPER, H, SUPER * 128))
    return {"xt": xt, "g": np.ascontiguousarray(gc), "ct2": ct2,
            "idp": idp, "ident": ident}


def run(x_u, C_weight, gumbel_noise, trace=False):
    from concourse.bass_utils import run_bass_kernel_spmd

    nc = build()
    xf = np.ascontiguousarray(x_u, dtype=np.float32).reshape(B * L, H)
    gf = np.ascontiguousarray(gumbel_noise, dtype=np.float32).reshape(B * L, A)
    ct2 = np.ascontiguousarray(
        np.asarray(C_weight, dtype=np.float32).T)   # [H, A]
    idp1 = (63 - np.arange(A, dtype=np.uint32))
    idp = np.ascontiguousarray(
        np.tile(idp1, SUPER)[None, :].repeat(128, axis=0))
    ident = np.eye(128, dtype=np.float32)

    in_maps = []
    for c in range(NCORES):
        sl = slice(c * TOK, (c + 1) * TOK)
        in_maps.append(_prep_core_inputs(xf[sl], gf[sl], ct2, idp, ident))
    res = run_bass_kernel_spmd(nc, in_maps, core_ids=list(range(NCORES)),
                               trace=trace)

    outs = np.stack([res.results[c]["out"] for c in range(NCORES)])
    # outs: [NCORES, NSUPER, 128, SUPER, 17]; token = s*1024 + p*8 + j
    ob = outs.reshape(NCORES * NSUPER * 128 * SUPER, 17)
    top8 = ob[:, 0:8]                        # stuffed f32, ranks 1-8
    sums = ob[:, 16]
    if SUPPRESS_MODE == "usub":
        sub8 = ob[:, 8:16].view(np.uint32)   # ranks 9-16, T-subtracted
        Tu = top8[:, 7:8].view(np.uint32)
        r916 = (sub8[:, 0:2] + Tu)           # ranks 9-10, stuffed bits
    else:
        r916 = ob[:, 8:10].view(np.uint32)   # ranks 9-10, raw stuffed bits
    b10 = np.concatenate([top8.view(np.uint32), r916], axis=1)   # [N, 10]
    idx = (63 - (b10 & 63)).astype(np.int32)
    w = (b10 & np.uint32(0xFFFFFFC0)).view(np.float32) / sums[:, None]
    w = w.reshape(B, L, K)
    idx = idx.reshape(B, L, K)
    return (w, idx), res


def kernel(x_u, C_weight, gumbel_noise):
    (w, i), _ = run(x_u, C_weight, gumbel_noise)
    return w, i
